# revision 1
# baseline (speedup 1.0000x reference)
"""Trainium2 Bass kernel for per-node LocalConv1D (kernel_size=1).

out[b, o, n] = sum_h W[n, o, h] * x[b, h, n] + b[n, o]

Full shapes: x [16, 32, 50000] f32, W [50000, 32, 32] f32, b [50000, 32] f32,
out [16, 32, 50000] f32.

Sharding: node dim n split evenly across 8 NeuronCores (6250 nodes/core,
zero-padded to 6272 = 49*128 inside each shard). Fully independent per-node
32x32 matmuls -> no collectives.

Per-core device strategy (memory-bound problem, ~52 MB of HBM traffic/core):
  - PE runs in 32x32 tiling mode (16 independent tiles). Tile (r, c) does
    one node's [32h x 32o] x [32h x 16b] matmul per instruction (f32
    self-loading stationary).
  - All operands are laid out in device DRAM by the host exactly as the
    engines consume them, so every DMA is a single large (>=0.5 MB)
    contiguous-run transfer: 4 x-loads, 13 W-loads, 1 bias load, 4 output
    stores per core. (The host-side shard prep already copies the arrays
    once; arranging the layout during that copy is free and keeps the
    device's HBM traffic at the minimum 52 MB/core.)
  - PSUM eviction is one DVE tensor_add per round which also adds the bias
    (resident in SBUF, broadcast over batch with a stride-0 AP dim).
  - The device writes its output in PE-native layout; the host un-permutes
    while assembling the full [16, 32, 50000] array.

Node bookkeeping per core (NPAD=6272 nodes): 4 PE row-quadrants r own
contiguous regions of Q=1568 nodes; within each quadrant, 4 PE columns c
own contiguous subregions of cc[c]*32 nodes (cc = [12,12,12,13]).
Round t (0..12): PE tile (r, c) processes the 32 nodes of chunk t of
column c (round 12: column 3 only).
"""

from contextlib import ExitStack

import numpy as np

import concourse.bass as bass
import concourse.mybir as mybir
import concourse.tile as tile
from concourse.ap import AP

F32 = mybir.dt.float32

B = 16  # batch
H = 32  # in channels
O = 32  # out channels
NCORES = 8
NFULL = 50000
NPC = NFULL // NCORES  # 6250 nodes per core
NPAD = 6272  # 49 * 128, per-core padded node count
Q = NPAD // 4  # 1568 nodes per row-quadrant
TOTAL_CHUNKS = NPAD // 128  # 49 chunks of 32 nodes per quadrant
CC0 = TOTAL_CHUNKS // 4  # 12 chunks per column for c < 3
CC3 = TOTAL_CHUNKS - 3 * CC0  # 13 chunks for column 3
CREG = CC0 * 32  # 384-node stride between column regions
CCM = CC3
E_ROUNDS = 4  # rounds per x/out pipeline group
N_GROUPS = (CC0 + E_ROUNDS - 1) // E_ROUNDS  # 3 full groups + tail round

# out DRAM slab layout: 3 group slabs of B*4*(32*E_ROUNDS) columns + tail slab
OUT_GF = B * 4 * 32 * E_ROUNDS  # 8192
OUT_TF = B * 4 * 32  # 2048
OUT_F = N_GROUPS * OUT_GF + (CC3 - CC0) * OUT_TF  # 26624


def _ap(handle_ap, offset, dims):
    """Raw AP on the same tensor: dims = [(step, count), ...] in elements."""
    return AP(handle_ap.tensor, offset, [[int(s), int(c)] for s, c in dims])


def build_bass():
    nc = bass.Bass()
    x_d = nc.declare_dram_parameter("x", [128, Q * B], F32, isOutput=False)
    w_d = nc.declare_dram_parameter("W", [CCM, 128, 4096], F32, isOutput=False)
    b_d = nc.declare_dram_parameter("b", [128, 4 * CCM * 32], F32, isOutput=False)
    out_d = nc.declare_dram_parameter("out", [128, OUT_F], F32, isOutput=True)

    with ExitStack() as ctx:
        tc = ctx.enter_context(tile.TileContext(nc))
        xp = ctx.enter_context(tc.tile_pool(name="xp", bufs=2))
        wtp = ctx.enter_context(tc.tile_pool(name="wtp", bufs=3))
        outp = ctx.enter_context(tc.tile_pool(name="outp", bufs=2))
        btp = ctx.enter_context(tc.tile_pool(name="btp", bufs=1))
        psp = ctx.enter_context(tc.tile_pool(name="psp", bufs=2, space="PSUM"))

        # resident bias [p=(c,o), f=(r:4, u:CCM*32)]
        bt = btp.tile([128, 4 * CCM * 32], F32)
        nc.sync.dma_start(out=bt[:], in_=b_d[:])

        for g in range(N_GROUPS):
            t0 = g * E_ROUNDS
            er = min(E_ROUNDS, CC0 - t0)
            gw = er * 32  # node window per (r, c)

            # x for this group: one DMA; tile f = (c:4, m:gw, b:16)
            x_t = xp.tile([128, 4 * gw * B], F32)
            src = _ap(
                x_d[:],
                t0 * 32 * B,
                [(Q * B, 128), (CREG * B, 4), (1, gw * B)],
            )
            nc.sync.dma_start(out=x_t[:], in_=src)

            # out accumulation tile f = (b:16, r:4, w:gw)
            out_t = outp.tile([128, B * 4 * gw], F32)

            for tl in range(er):
                t = t0 + tl
                # W for round t, already transposed: [p=(r,h), f=(c,o,j)]
                wt = wtp.tile([128, 4096], F32)
                nc.sync.dma_start(out=wt[:], in_=w_d[t])

                ps = psp.tile([128, 2048], F32)  # f=(r:4, j:32, b:16)
                wt_v = wt[:].rearrange("p (c o j) -> p c o j", c=4, o=O, j=32)
                x_v = x_t[:].rearrange("p (c m b) -> p c m b", c=4, m=gw, b=B)
                ps_v = ps[:].rearrange("p (r j b) -> p r j b", r=4, j=32, b=B)
                for j in range(32):
                    for r in range(4):
                        for c in range(4):
                            nc.tensor.matmul(
                                ps_v[32 * c : 32 * c + 32, r, j, :],
                                wt_v[32 * r : 32 * r + 32, c, :, j],
                                x_v[32 * r : 32 * r + 32, c, tl * 32 + j, :],
                                start=True,
                                stop=True,
                                tile_position=(32 * r, 32 * c),
                            )

                # eviction + bias add (DVE), iter (r, j, b)
                out_ap = (
                    out_t[:]
                    .rearrange("p (b r w) -> p b r w", b=B, r=4, w=gw)[
                        :, :, :, tl * 32 : tl * 32 + 32
                    ]
                    .transpose([0, 2, 3, 1])
                )
                bt_ap = (
                    bt[:]
                    .rearrange("p (r u) -> p r u", r=4)[:, :, t * 32 : t * 32 + 32]
                    .unsqueeze(3)
                    .broadcast_to([128, 4, 32, B])
                )
                nc.vector.tensor_add(out_ap, ps_v[:, :, :, :], bt_ap)

            # one output DMA for the group
            nc.scalar.dma_start(
                out=out_d[:, g * OUT_GF : g * OUT_GF + B * 4 * gw],
                in_=out_t[:],
            )

        # ---- tail rounds: column c=3 only ----
        for t in range(CC0, CC3):
            ti = t - CC0
            x3 = xp.tile([128, 32 * B], F32, tag="x_t")
            src = _ap(
                x_d[:],
                (3 * CREG + t * 32) * B,
                [(Q * B, 128), (1, 32 * B)],
            )
            nc.sync.dma_start(out=x3[:], in_=src)

            wt = wtp.tile([128, 1024], F32, tag="wtp")
            nc.sync.dma_start(out=wt[:], in_=w_d[t, :, 3 * 1024 : 4 * 1024])

            ps = psp.tile([128, 2048], F32)
            wt_v = wt[:].rearrange("p (o j) -> p o j", o=O, j=32)
            x_v = x3[:].rearrange("p (m b) -> p m b", m=32, b=B)
            ps_v = ps[:].rearrange("p (r j b) -> p r j b", r=4, j=32, b=B)
            for j in range(32):
                for r in range(4):
                    nc.tensor.matmul(
                        ps_v[96:128, r, j, :],
                        wt_v[32 * r : 32 * r + 32, :, j],
                        x_v[32 * r : 32 * r + 32, j, :],
                        start=True,
                        stop=True,
                        tile_position=(32 * r, 96),
                    )

            out3 = outp.tile([128, 2048], F32, tag="out_t")  # f=(b,r,w:32)
            out_ap = (
                out3[96:128]
                .rearrange("p (b r w) -> p b r w", b=B, r=4, w=32)
                .transpose([0, 2, 3, 1])
            )
            bt_ap = (
                bt[96:128]
                .rearrange("p (r u) -> p r u", r=4)[:, :, t * 32 : t * 32 + 32]
                .unsqueeze(3)
                .broadcast_to([32, 4, 32, B])
            )
            nc.vector.tensor_add(out_ap, ps_v[96:128, :, :, :], bt_ap)

            nc.scalar.dma_start(
                out=out_d[
                    96:128,
                    N_GROUPS * OUT_GF + ti * OUT_TF : N_GROUPS * OUT_GF
                    + (ti + 1) * OUT_TF,
                ],
                in_=out3[96:128],
            )

    return nc


def _legalize_waits(nc):
    """Walrus's per-instruction sync structs carry at most one wait
    (DMA_DIRECT2D, S3_LW, ...); Tile sometimes leaves several on one
    instruction. Move the surplus onto EventSemaphore instructions inserted
    just before it on the same engine — the issuing sequencer executes its
    stream in order, so the waits still gate the instruction."""
    nsplit = 0
    for f in nc.m.functions:
        for bb in f.blocks:
            new = []
            changed = False
            for inst in bb.instructions:
                si = getattr(inst, "sync_info", None)
                if (
                    si is not None
                    and si.on_wait
                    and len(si.on_wait) > 1
                    and type(inst).__name__ != "InstEventSemaphore"
                ):
                    waits = list(si.on_wait)
                    for w in waits[:-1]:
                        nsplit += 1
                        new.append(
                            mybir.InstEventSemaphore(
                                name=f"wait-split-{nsplit}",
                                engine=inst.engine,
                                ins=[],
                                outs=[],
                                sync_info=mybir.SyncInfo(
                                    on_wait=[w], on_update=[]
                                ),
                            )
                        )
                    inst.sync_info = mybir.SyncInfo(
                        on_wait=[waits[-1]], on_update=list(si.on_update)
                    )
                    changed = True
                new.append(inst)
            if changed:
                bb.instructions = new
    return nc


_NC_CACHE = {}


def _get_nc():
    if "nc" not in _NC_CACHE:
        _NC_CACHE["nc"] = _legalize_waits(build_bass())
    return _NC_CACHE["nc"]


# column-region offsets within a quadrant and chunks per column
_CRE = [0, CREG, 2 * CREG, 3 * CREG]
_CCS = [CC0, CC0, CC0, CC3]


def prep_core_inputs(x_s, W_s, b_s):
    """Per-core shard [*, NPC nodes] -> device-layout arrays (padded)."""
    xs = np.zeros((B, H, NPAD), np.float32)
    xs[:, :, :NPC] = x_s
    Ws = np.zeros((NPAD, O, H), np.float32)
    Ws[:NPC] = W_s
    bs = np.zeros((NPAD, O), np.float32)
    bs[:NPC] = b_s

    # x: [p=(r,h), f=(m,b)] ; m is the node index within the quadrant
    xp = (
        xs.reshape(B, H, 4, Q)
        .transpose(2, 1, 3, 0)
        .reshape(128, Q * B)
        .copy()
    )

    # W: [t, p=(r,h), f=(c,o,j)], pre-transposed per node
    wp = np.zeros((CCM, 128, 4096), np.float32)
    W4 = Ws.reshape(4, Q, O, H)
    for c in range(4):
        nch = _CCS[c]
        Wc = W4[:, _CRE[c] : _CRE[c] + nch * 32].reshape(4, nch, 32, O, H)
        # -> [t, (r,h), (o,j)]
        wp[:nch, :, c * 1024 : (c + 1) * 1024] = (
            Wc.transpose(1, 0, 4, 3, 2).reshape(nch, 128, 1024)
        )

    # bias: [p=(c,o), f=(r, u:CCM*32)]
    bp = np.zeros((128, 4 * CCM * 32), np.float32)
    b4 = bs.reshape(4, Q, O)
    for c in range(4):
        nch = _CCS[c]
        for r in range(4):
            bc = b4[r, _CRE[c] : _CRE[c] + nch * 32]  # [len, O]
            bp[c * 32 : (c + 1) * 32, r * CCM * 32 : r * CCM * 32 + nch * 32] = (
                bc.T
            )

    return {"x": xp, "W": wp, "b": bp}


def unprep_core_output(op):
    """Device out slab [128, OUT_F] -> [B, O, NPC]."""
    out = np.empty((B, O, NPAD), np.float32)
    o4 = out.reshape(B, O, 4, Q)
    for g in range(N_GROUPS):
        gw = min(E_ROUNDS, CC0 - g * E_ROUNDS) * 32
        slab = op[:, g * OUT_GF : g * OUT_GF + B * 4 * gw].reshape(
            4, 32, B, 4, gw
        )
        for c in range(4):
            w0 = _CRE[c] + g * E_ROUNDS * 32
            o4[:, :, :, w0 : w0 + gw] = slab[c].transpose(1, 0, 2, 3)
    for t in range(CC0, CC3):
        ti = t - CC0
        slab = op[
            96:128,
            N_GROUPS * OUT_GF + ti * OUT_TF : N_GROUPS * OUT_GF
            + (ti + 1) * OUT_TF,
        ].reshape(32, B, 4, 32)
        o4[:, :, :, _CRE[3] + t * 32 : _CRE[3] + t * 32 + 32] = slab.transpose(
            1, 0, 2, 3
        )
    return out[:, :, :NPC]


def make_in_maps(x, W, b):
    x = np.ascontiguousarray(x, dtype=np.float32)
    W = np.ascontiguousarray(W, dtype=np.float32)
    b = np.ascontiguousarray(b, dtype=np.float32)
    in_maps = []
    for core in range(NCORES):
        sl = slice(core * NPC, (core + 1) * NPC)
        in_maps.append(
            prep_core_inputs(x[:, :, sl], W[sl], b[sl])
        )
    return in_maps


def run_spmd(in_maps, **kwargs):
    from concourse.bass_utils import run_bass_kernel_spmd

    nc = _get_nc()
    return run_bass_kernel_spmd(
        nc, in_maps, core_ids=list(range(NCORES)), **kwargs
    )


def kernel(x, W, b):
    res = run_spmd(make_in_maps(x, W, b))
    out = np.concatenate(
        [unprep_core_output(res.results[c]["out"]) for c in range(NCORES)],
        axis=2,
    )
    return out



# revision 2
# speedup vs baseline: 1.8520x; 1.8520x over previous
"""Trainium2 Bass kernel for per-node LocalConv1D (kernel_size=1).

out[b, o, n] = sum_h W[n, o, h] * x[b, h, n] + b[n, o]

Full shapes: x [16, 32, 50000] f32, W [50000, 32, 32] f32, b [50000, 32] f32,
out [16, 32, 50000] f32.

Sharding: node dim n split evenly across 8 NeuronCores (6250 nodes/core,
zero-padded to 6272 = 49*128 inside each shard). Fully independent per-node
32x32 matmuls -> no collectives.

Per-core device strategy (memory-bound problem, ~52 MB of HBM traffic/core):
  - PE runs in 32x32 tiling mode (16 independent tiles). Tile (r, c) does
    one node's [32h x 32o] x [32h x 16b] matmul per instruction (f32
    self-loading stationary).
  - All operands are laid out in device DRAM by the host exactly as the
    engines consume them, so every DMA is a single large (>=0.5 MB)
    contiguous-run transfer: 4 x-loads, 13 W-loads, 1 bias load, 4 output
    stores per core. (The host-side shard prep already copies the arrays
    once; arranging the layout during that copy is free and keeps the
    device's HBM traffic at the minimum 52 MB/core.)
  - PSUM eviction is one DVE tensor_add per round which also adds the bias
    (resident in SBUF, broadcast over batch with a stride-0 AP dim).
  - The device writes its output in PE-native layout; the host un-permutes
    while assembling the full [16, 32, 50000] array.

Node bookkeeping per core (NPAD=6272 nodes): 4 PE row-quadrants r own
contiguous regions of Q=1568 nodes; within each quadrant, 4 PE columns c
own contiguous subregions of cc[c]*32 nodes (cc = [12,12,12,13]).
Round t (0..12): PE tile (r, c) processes the 32 nodes of chunk t of
column c (round 12: column 3 only).
"""

from contextlib import ExitStack

import numpy as np

import concourse.bass as bass
import concourse.mybir as mybir
import concourse.tile as tile
from concourse.ap import AP

F32 = mybir.dt.float32
BF16 = mybir.dt.bfloat16

B = 16  # batch
H = 32  # in channels
O = 32  # out channels
NCORES = 8
NFULL = 50000
NPC = NFULL // NCORES  # 6250 nodes per core
NPAD = 6272  # 49 * 128, per-core padded node count
Q = NPAD // 4  # 1568 nodes per row-quadrant
TOTAL_CHUNKS = NPAD // 128  # 49 chunks of 32 nodes per quadrant
CC0 = TOTAL_CHUNKS // 4  # 12 chunks per column for c < 3
CC3 = TOTAL_CHUNKS - 3 * CC0  # 13 chunks for column 3
CREG = CC0 * 32  # 384-node stride between column regions
CCM = CC3
E_ROUNDS = 4  # rounds per x/out pipeline group
N_GROUPS = (CC0 + E_ROUNDS - 1) // E_ROUNDS  # 3 full groups + tail round

# out DRAM slab layout: 3 group slabs of B*4*(32*E_ROUNDS) columns + tail slab
OUT_GF = B * 4 * 32 * E_ROUNDS  # 8192
OUT_TF = B * 4 * 32  # 2048
OUT_F = N_GROUPS * OUT_GF + (CC3 - CC0) * OUT_TF  # 26624


def _ap(handle_ap, offset, dims):
    """Raw AP on the same tensor: dims = [(step, count), ...] in elements."""
    return AP(handle_ap.tensor, offset, [[int(s), int(c)] for s, c in dims])


def build_bass():
    nc = bass.Bass()
    x_d = nc.declare_dram_parameter("x", [128, Q * B], BF16, isOutput=False)
    w_d = nc.declare_dram_parameter("W", [CCM, 128, 4096], BF16, isOutput=False)
    b_d = nc.declare_dram_parameter("b", [128, 4 * CCM * 32], F32, isOutput=False)
    out_d = nc.declare_dram_parameter("out", [128, OUT_F], BF16, isOutput=True)

    with ExitStack() as ctx:
        tc = ctx.enter_context(tile.TileContext(nc))
        xp = ctx.enter_context(tc.tile_pool(name="xp", bufs=2))
        wtp = ctx.enter_context(tc.tile_pool(name="wtp", bufs=3))
        outp = ctx.enter_context(tc.tile_pool(name="outp", bufs=2))
        btp = ctx.enter_context(tc.tile_pool(name="btp", bufs=1))
        psp = ctx.enter_context(tc.tile_pool(name="psp", bufs=2, space="PSUM"))

        # resident bias [p=(c,o), f=(r:4, u:CCM*32)]
        bt = btp.tile([128, 4 * CCM * 32], F32)
        nc.sync.dma_start(out=bt[:], in_=b_d[:])

        for g in range(N_GROUPS):
            t0 = g * E_ROUNDS
            er = min(E_ROUNDS, CC0 - t0)
            gw = er * 32  # node window per (r, c)

            # x for this group: one DMA; tile f = (c:4, m:gw, b:16)
            x_t = xp.tile([128, 4 * gw * B], BF16)
            src = _ap(
                x_d[:],
                t0 * 32 * B,
                [(Q * B, 128), (CREG * B, 4), (1, gw * B)],
            )
            nc.sync.dma_start(out=x_t[:], in_=src)

            # out accumulation tile f = (b:16, r:4, w:gw)
            out_t = outp.tile([128, B * 4 * gw], BF16)

            for tl in range(er):
                t = t0 + tl
                # W for round t, already transposed: [p=(r,h), f=(c,o,j)]
                wt = wtp.tile([128, 4096], BF16)
                nc.sync.dma_start(out=wt[:], in_=w_d[t])

                ps = psp.tile([128, 2048], F32)  # f=(r:4, j:32, b:16)
                wt_v = wt[:].rearrange("p (c o j) -> p c o j", c=4, o=O, j=32)
                x_v = x_t[:].rearrange("p (c m b) -> p c m b", c=4, m=gw, b=B)
                ps_v = ps[:].rearrange("p (r j b) -> p r j b", r=4, j=32, b=B)
                for j in range(32):
                    for r in range(4):
                        for c in range(4):
                            nc.tensor.matmul(
                                ps_v[32 * c : 32 * c + 32, r, j, :],
                                wt_v[32 * r : 32 * r + 32, c, :, j],
                                x_v[32 * r : 32 * r + 32, c, tl * 32 + j, :],
                                start=True,
                                stop=True,
                                tile_position=(32 * r, 32 * c),
                            )

                # eviction + bias add (DVE), iter (r, j, b)
                out_ap = (
                    out_t[:]
                    .rearrange("p (b r w) -> p b r w", b=B, r=4, w=gw)[
                        :, :, :, tl * 32 : tl * 32 + 32
                    ]
                    .transpose([0, 2, 3, 1])
                )
                bt_ap = (
                    bt[:]
                    .rearrange("p (r u) -> p r u", r=4)[:, :, t * 32 : t * 32 + 32]
                    .unsqueeze(3)
                    .broadcast_to([128, 4, 32, B])
                )
                nc.vector.tensor_add(out_ap, ps_v[:, :, :, :], bt_ap)

            # one output DMA for the group
            nc.scalar.dma_start(
                out=out_d[:, g * OUT_GF : g * OUT_GF + B * 4 * gw],
                in_=out_t[:],
            )

        # ---- tail rounds: column c=3 only ----
        for t in range(CC0, CC3):
            ti = t - CC0
            x3 = xp.tile([128, 32 * B], BF16, tag="x_t")
            src = _ap(
                x_d[:],
                (3 * CREG + t * 32) * B,
                [(Q * B, 128), (1, 32 * B)],
            )
            nc.sync.dma_start(out=x3[:], in_=src)

            wt = wtp.tile([128, 1024], BF16, tag="wtp")
            nc.sync.dma_start(out=wt[:], in_=w_d[t, :, 3 * 1024 : 4 * 1024])

            ps = psp.tile([128, 2048], F32)
            wt_v = wt[:].rearrange("p (o j) -> p o j", o=O, j=32)
            x_v = x3[:].rearrange("p (m b) -> p m b", m=32, b=B)
            ps_v = ps[:].rearrange("p (r j b) -> p r j b", r=4, j=32, b=B)
            for j in range(32):
                for r in range(4):
                    nc.tensor.matmul(
                        ps_v[96:128, r, j, :],
                        wt_v[32 * r : 32 * r + 32, :, j],
                        x_v[32 * r : 32 * r + 32, j, :],
                        start=True,
                        stop=True,
                        tile_position=(32 * r, 96),
                    )

            out3 = outp.tile([128, 2048], BF16, tag="out_t")  # f=(b,r,w:32)
            out_ap = (
                out3[96:128]
                .rearrange("p (b r w) -> p b r w", b=B, r=4, w=32)
                .transpose([0, 2, 3, 1])
            )
            bt_ap = (
                bt[96:128]
                .rearrange("p (r u) -> p r u", r=4)[:, :, t * 32 : t * 32 + 32]
                .unsqueeze(3)
                .broadcast_to([32, 4, 32, B])
            )
            nc.vector.tensor_add(out_ap, ps_v[96:128, :, :, :], bt_ap)

            nc.scalar.dma_start(
                out=out_d[
                    96:128,
                    N_GROUPS * OUT_GF + ti * OUT_TF : N_GROUPS * OUT_GF
                    + (ti + 1) * OUT_TF,
                ],
                in_=out3[96:128],
            )

    return nc


def _legalize_waits(nc):
    """Walrus's per-instruction sync structs carry at most one wait
    (DMA_DIRECT2D, S3_LW, ...); Tile sometimes leaves several on one
    instruction. Move the surplus onto EventSemaphore instructions inserted
    just before it on the same engine — the issuing sequencer executes its
    stream in order, so the waits still gate the instruction."""
    nsplit = 0
    for f in nc.m.functions:
        for bb in f.blocks:
            new = []
            changed = False
            for inst in bb.instructions:
                si = getattr(inst, "sync_info", None)
                if (
                    si is not None
                    and si.on_wait
                    and len(si.on_wait) > 1
                    and type(inst).__name__ != "InstEventSemaphore"
                ):
                    waits = list(si.on_wait)
                    for w in waits[:-1]:
                        nsplit += 1
                        new.append(
                            mybir.InstEventSemaphore(
                                name=f"wait-split-{nsplit}",
                                engine=inst.engine,
                                ins=[],
                                outs=[],
                                sync_info=mybir.SyncInfo(
                                    on_wait=[w], on_update=[]
                                ),
                            )
                        )
                    inst.sync_info = mybir.SyncInfo(
                        on_wait=[waits[-1]], on_update=list(si.on_update)
                    )
                    changed = True
                new.append(inst)
            if changed:
                bb.instructions = new
    return nc


_NC_CACHE = {}


def _get_nc():
    if "nc" not in _NC_CACHE:
        _NC_CACHE["nc"] = _legalize_waits(build_bass())
    return _NC_CACHE["nc"]


# column-region offsets within a quadrant and chunks per column
_CRE = [0, CREG, 2 * CREG, 3 * CREG]
_CCS = [CC0, CC0, CC0, CC3]


def prep_core_inputs(x_s, W_s, b_s):
    """Per-core shard [*, NPC nodes] -> device-layout arrays (padded)."""
    import ml_dtypes
    bf16 = ml_dtypes.bfloat16
    xs = np.zeros((B, H, NPAD), bf16)
    xs[:, :, :NPC] = x_s.astype(bf16)
    Ws = np.zeros((NPAD, O, H), bf16)
    Ws[:NPC] = W_s.astype(bf16)
    bs = np.zeros((NPAD, O), np.float32)
    bs[:NPC] = b_s

    # x: [p=(r,h), f=(m,b)] ; m is the node index within the quadrant
    xp = (
        xs.reshape(B, H, 4, Q)
        .transpose(2, 1, 3, 0)
        .reshape(128, Q * B)
        .copy()
    )

    # W: [t, p=(r,h), f=(c,o,j)], pre-transposed per node
    wp = np.zeros((CCM, 128, 4096), bf16)
    W4 = Ws.reshape(4, Q, O, H)
    for c in range(4):
        nch = _CCS[c]
        Wc = W4[:, _CRE[c] : _CRE[c] + nch * 32].reshape(4, nch, 32, O, H)
        # -> [t, (r,h), (o,j)]
        wp[:nch, :, c * 1024 : (c + 1) * 1024] = (
            Wc.transpose(1, 0, 4, 3, 2).reshape(nch, 128, 1024)
        )

    # bias: [p=(c,o), f=(r, u:CCM*32)]
    bp = np.zeros((128, 4 * CCM * 32), np.float32)
    b4 = bs.reshape(4, Q, O)
    for c in range(4):
        nch = _CCS[c]
        for r in range(4):
            bc = b4[r, _CRE[c] : _CRE[c] + nch * 32]  # [len, O]
            bp[c * 32 : (c + 1) * 32, r * CCM * 32 : r * CCM * 32 + nch * 32] = (
                bc.T
            )

    return {"x": xp, "W": wp, "b": bp}


def unprep_core_output(op):
    """Device out slab [128, OUT_F] -> [B, O, NPC]."""
    op = np.asarray(op).astype(np.float32)
    out = np.empty((B, O, NPAD), np.float32)
    o4 = out.reshape(B, O, 4, Q)
    for g in range(N_GROUPS):
        gw = min(E_ROUNDS, CC0 - g * E_ROUNDS) * 32
        slab = op[:, g * OUT_GF : g * OUT_GF + B * 4 * gw].reshape(
            4, 32, B, 4, gw
        )
        for c in range(4):
            w0 = _CRE[c] + g * E_ROUNDS * 32
            o4[:, :, :, w0 : w0 + gw] = slab[c].transpose(1, 0, 2, 3)
    for t in range(CC0, CC3):
        ti = t - CC0
        slab = op[
            96:128,
            N_GROUPS * OUT_GF + ti * OUT_TF : N_GROUPS * OUT_GF
            + (ti + 1) * OUT_TF,
        ].reshape(32, B, 4, 32)
        o4[:, :, :, _CRE[3] + t * 32 : _CRE[3] + t * 32 + 32] = slab.transpose(
            1, 0, 2, 3
        )
    return out[:, :, :NPC]


def make_in_maps(x, W, b):
    x = np.ascontiguousarray(x, dtype=np.float32)
    W = np.ascontiguousarray(W, dtype=np.float32)
    b = np.ascontiguousarray(b, dtype=np.float32)
    in_maps = []
    for core in range(NCORES):
        sl = slice(core * NPC, (core + 1) * NPC)
        in_maps.append(
            prep_core_inputs(x[:, :, sl], W[sl], b[sl])
        )
    return in_maps


def run_spmd(in_maps, **kwargs):
    from concourse.bass_utils import run_bass_kernel_spmd

    nc = _get_nc()
    return run_bass_kernel_spmd(
        nc, in_maps, core_ids=list(range(NCORES)), **kwargs
    )


def kernel(x, W, b):
    res = run_spmd(make_in_maps(x, W, b))
    out = np.concatenate(
        [unprep_core_output(res.results[c]["out"]) for c in range(NCORES)],
        axis=2,
    )
    return out



# revision 20
# speedup vs baseline: 2.7409x; 1.4799x over previous
"""Trainium2 Bass kernel for per-node LocalConv1D (kernel_size=1).

out[b, o, n] = sum_h W[n, o, h] * x[b, h, n] + b[n, o]

Full shapes: x [16, 32, 50000] f32, W [50000, 32, 32] f32, b [50000, 32] f32,
out [16, 32, 50000] f32.

Sharding: node dim n split evenly across 8 NeuronCores (6250 nodes/core,
zero-padded to 6272 = 98*64 inside each shard). Fully independent per-node
32x32 matmuls -> no collectives.

Device strategy (v2, x-stationary):
  The PE weight path (LDWEIGHTS) costs ~1 cycle per stationary COLUMN, so we
  make x the stationary operand: one LDWEIGHTS of [32h x 32] covers the
  16-batch columns of TWO nodes (a "pair"), and each pair is ONE InstMatmult
  whose moving operand is the two nodes' weights [32h x 64o] streamed from
  SBUF on the other read port. All tensors are bf16 (tolerance 2e-2; bf16
  gives ~4e-3). PSUM result per pair is [32 x 64] where for node i of the
  pair only rows i*16..i*16+16 are valid (the other 16 rows are the cross
  product of the wrong node's x and are discarded on the host).

  PE 32x32 tiling: row group r = node's quadrant (its h rows live on
  partitions 32r..32r+32 of the x/W SBUF layout), col group c rotates over
  pairs so PSUM fills all 128 partitions. A round = 64 nodes (16 per
  quadrant) = 32 matmuls = one PSUM bank [128 x 512 f32]; 98 rounds total.
  Eviction is a dense PSUM->SBUF bf16 copy (garbage included), alternating
  DVE / ACT engines; out DMA ships dense bf16 slabs, host strips garbage.

  Bias is added on the host during reassembly (out partitions are batch
  lanes on-device, so a device-side bias add would need a partition
  broadcast the vector engines don't have).
"""

from contextlib import ExitStack

import numpy as np

import concourse.bass as bass
import concourse.mybir as mybir
import concourse.tile as tile

F32 = mybir.dt.float32
BF16 = mybir.dt.bfloat16

B = 16  # batch
H = 32  # in channels
O = 32  # out channels
NCORES = 8
NFULL = 50000
NPC = NFULL // NCORES  # 6250 nodes per core
NPAD = 6272  # 98 * 64, per-core padded node count
Q = NPAD // 4  # 1568 nodes per quadrant (row group)
NPR = 16  # nodes per quadrant per round
ROUNDS = Q // NPR  # 98
RPS = 7  # rounds per W/out slab
NSLAB = ROUNDS // RPS  # 14
RPX = 14  # rounds per x chunk
NXT = ROUNDS // RPX  # 7 x chunks
XF = RPX * NPR * B  # 3584 x f-columns per chunk
WF = RPS * NPR * O  # 3584 W f-columns per slab
OF = RPS * 512  # 3584 out f-columns per slab
OUT_F = ROUNDS * 512  # 50176


def build_bass():
    import os
    nslab_run = int(os.environ.get("NSLAB_RUN", NSLAB))
    nxt_run = int(os.environ.get("NXT_RUN", NXT))
    nc = bass.Bass()
    x_d = nc.declare_dram_parameter("x", [128, Q * B], BF16, isOutput=False)
    w_d = nc.declare_dram_parameter("W", [128, Q * O], BF16, isOutput=False)
    out_d = nc.declare_dram_parameter("out", [128, OUT_F], BF16, isOutput=True)

    with ExitStack() as ctx:
        tc = ctx.enter_context(tile.TileContext(nc))
        xtp = ctx.enter_context(tc.tile_pool(name="xtp", bufs=NXT))
        wtp = ctx.enter_context(tc.tile_pool(name="wtp", bufs=3))
        outp = ctx.enter_context(tc.tile_pool(name="outp", bufs=3))
        psp = ctx.enter_context(tc.tile_pool(name="psp", bufs=4, space="PSUM"))

        # resident x chunks; first chunk and first W slab race in parallel
        xts = []
        for t in range(nxt_run):
            xt = xtp.tile([128, XF], BF16)
            nc.sync.dma_start(out=xt[:], in_=x_d[:, t * XF : (t + 1) * XF])
            xts.append(xt)

        for sl in range(nslab_run):
            wt = wtp.tile([128, WF], BF16)
            nc.sync.dma_start(out=wt[:], in_=w_d[:, sl * WF : (sl + 1) * WF])
            ot = outp.tile([128, OF], BF16)

            for lr in range(RPS):
                g = sl * RPS + lr
                xt = xts[min(g // RPX, nxt_run - 1)]
                xbase = (g % RPX) * NPR * B  # start f of this round's nodes
                ps = psp.tile([128, 512], F32)
                ps_v = ps[:]
                x_v = xt[:]
                w_v = wt[:]
                for k in range(8):
                    for r in range(4):
                        nc.tensor.matmul(
                            ps_v[32 * r : 32 * r + 32, k * 64 : k * 64 + 64],
                            x_v[32 * r : 32 * r + 32,
                                xbase + k * 2 * B : xbase + k * 2 * B + 2 * B],
                            w_v[32 * r : 32 * r + 32,
                                lr * 512 + k * 2 * O : lr * 512 + k * 2 * O + 2 * O],
                            start=True,
                            stop=True,
                            tile_position=(32 * r, 32 * r),
                        )
                dst = ot[:][:, lr * 512 : (lr + 1) * 512]
                nc.vector.tensor_copy(out=dst, in_=ps_v)

            nc.scalar.dma_start(
                out=out_d[:, sl * OF : (sl + 1) * OF], in_=ot[:]
            )

    return nc


def _legalize_waits(nc):
    """Walrus's per-instruction sync structs carry at most one wait
    (DMA_DIRECT2D, S3_LW, ...); Tile sometimes leaves several on one
    instruction. Move the surplus onto EventSemaphore instructions inserted
    just before it on the same engine — the issuing sequencer executes its
    stream in order, so the waits still gate the instruction."""
    nsplit = 0
    for f in nc.m.functions:
        for bb in f.blocks:
            new = []
            changed = False
            for inst in bb.instructions:
                si = getattr(inst, "sync_info", None)
                if (
                    si is not None
                    and si.on_wait
                    and len(si.on_wait) > 1
                    and type(inst).__name__ != "InstEventSemaphore"
                ):
                    waits = list(si.on_wait)
                    for w in waits[:-1]:
                        nsplit += 1
                        new.append(
                            mybir.InstEventSemaphore(
                                name=f"wait-split-{nsplit}",
                                engine=inst.engine,
                                ins=[],
                                outs=[],
                                sync_info=mybir.SyncInfo(
                                    on_wait=[w], on_update=[]
                                ),
                            )
                        )
                    inst.sync_info = mybir.SyncInfo(
                        on_wait=[waits[-1]], on_update=list(si.on_update)
                    )
                    changed = True
                new.append(inst)
            if changed:
                bb.instructions = new
    return nc


_NC_CACHE = {}


def _get_nc():
    if "nc" not in _NC_CACHE:
        _NC_CACHE["nc"] = _legalize_waits(build_bass())
    return _NC_CACHE["nc"]


def prep_core_inputs(x_s, W_s):
    """Per-core shard [*, NPC nodes] -> device-layout bf16 arrays (padded)."""
    import ml_dtypes

    bf16 = ml_dtypes.bfloat16
    xs = np.zeros((B, H, NPAD), bf16)
    xs[:, :, :NPC] = x_s.astype(bf16)
    Ws = np.zeros((NPAD, O, H), bf16)
    Ws[:NPC] = W_s.astype(bf16)

    # x: [p=(r,h), f=(m,b)] ; m is the node index within the quadrant
    xp = (
        xs.reshape(B, H, 4, Q)
        .transpose(2, 1, 3, 0)
        .reshape(128, Q * B)
        .copy()
    )

    # W: [p=(r,h), f=(m,o)] (per-node W transposed to [h, o])
    wp = (
        Ws.reshape(4, Q, O, H)
        .transpose(0, 3, 1, 2)
        .reshape(128, Q * O)
        .copy()
    )

    return {"x": xp, "W": wp}


def unprep_core_output(op):
    """Device out slab [128, OUT_F] bf16 -> [B, O, NPC] f32 (garbage rows
    of each pair stripped)."""
    arr = np.asarray(op).astype(np.float32)
    # p = (r:4, i:2, b:16), f = (g:98, k:8, i:2, o:32)
    arr = arr.reshape(4, 2, B, ROUNDS, 8, 2, O)
    diag = arr[:, [0, 1], :, :, :, [0, 1], :]  # [i, r, b, g, k, o]
    # node n = r*Q + g*16 + k*2 + i -> order [r, g, k, i]
    out = diag.transpose(2, 5, 1, 3, 4, 0).reshape(B, O, NPAD)
    return out[:, :, :NPC]


def make_in_maps(x, W, b=None):
    x = np.ascontiguousarray(x, dtype=np.float32)
    W = np.ascontiguousarray(W, dtype=np.float32)
    in_maps = []
    for core in range(NCORES):
        sl = slice(core * NPC, (core + 1) * NPC)
        in_maps.append(prep_core_inputs(x[:, :, sl], W[sl]))
    return in_maps


def run_spmd(in_maps, **kwargs):
    from concourse.bass_utils import run_bass_kernel_spmd

    nc = _get_nc()
    return run_bass_kernel_spmd(
        nc, in_maps, core_ids=list(range(NCORES)), **kwargs
    )


def assemble_output(res, b):
    out = np.concatenate(
        [unprep_core_output(res.results[c]["out"]) for c in range(NCORES)],
        axis=2,
    )
    # bias epilogue on host: out[b, o, n] += bias[n, o]
    out += np.ascontiguousarray(b, dtype=np.float32).T[None, :, :]
    return out


def kernel(x, W, b):
    res = run_spmd(make_in_maps(x, W))
    return assemble_output(res, b)
